# revision 47
# baseline (speedup 1.0000x reference)
"""Trainium2 Bass kernel for an 8-layer densely-connected MLP.

Math: the reference's dense past/future skip-connection structure is linear
in the per-layer silu outputs a_i, so it collapses (host-side, fp64) to

    a_0 = silu(x @ W0.T + b0)
    a_i = silu(sum_{m<i} a_m @ P[i][m].T + bh[i-1])      i = 1..7
    out = log_softmax(a_7 @ Wout.T + bout)

with 28 precomputed 64x64 matrices P[i][m].

Device layout: activations are feature-major ([64 feat, N batch] tiles).
x.T is pre-transposed and cast to fp16 on the host so the device only ever
does full-rate contiguous DMA loads.  Each megatile is 2048 batch rows,
processed as two 1024-row "chunks" living in partition halves 0:64 / 64:128;
the two chunks run as concurrent 2D-tiled matmuls on the PE array
(tile_position row/col groups), and one [128, 512] Silu activation op covers
both chunks at full lane utilization.  Raw logits are PE-transposed to
batch-major and log-softmax runs once at the end (single ACT table switch).
"""

import sys

sys.path.insert(0, "/opt/trn_rl_repo")

import numpy as np

from contextlib import ExitStack

from concourse import bass, mybir, tile
from concourse.bass_utils import run_bass_kernel_spmd

# Problem constants (hardcoded per harness contract)
B, IN, H, OUT, L = 65536, 784, 64, 10, 8
N_CORES = 8
B_CORE = B // N_CORES            # 8192
KBLK = 7                         # K blocks for layer 0
KP = 128                         # padded K-block height (784 -> 7*128, zero-padded)
NPAIR = L * (L - 1) // 2         # 28 (i,m) weight blocks
MAXCHUNK = 1024

# Megatile sizes ramp up so PE compute starts before much of x has streamed
# in (the DMA prime would otherwise idle PE for ~25us at full-size tiles).
MEGA_SCHED = [512, 512, 512, 512, 1024, 1024, 2048, 2048]
assert sum(MEGA_SCHED) == B_CORE


def make_sched(sizes):
    """Per-megatile metadata: batch start, chunk width, transpose blocks,
    out_acc column offset."""
    meta, start, aoff = [], 0, 0
    for mg, mega in enumerate(sizes):
        nblk = mega // 128
        meta.append(
            dict(mg=mg, mega=mega, start=start, chunk=mega // 2, nblk=nblk, aoff=aoff)
        )
        start += mega
        aoff += nblk * OUT
    return meta


SCHED = make_sched(MEGA_SCHED)
ACOLS = sum(m["nblk"] * OUT for m in SCHED)  # 640
NBLK_MAX = max(m["nblk"] for m in SCHED)     # 16
XCOLS = KBLK * sum(m["mega"] for m in SCHED)  # x.T columns per core, tiled layout


def _xoff(sched):
    """Column offset of each (mg, ck) slab in the per-core x tensor.
    Layout: [128 partitions, Σ_mg Σ_ck KBLK*chunk] — per-partition contiguous
    so every load is 128 maximal-length DMA descriptors."""
    offs, off = {}, 0
    for m in sched:
        for ck in range(2):
            offs[(m["mg"], ck)] = off
            off += KBLK * m["chunk"]
    return offs, off

f16 = mybir.dt.float16
f32 = mybir.dt.float32
AF = mybir.ActivationFunctionType


# ----------------------------------------------------------------------------
# Host-side weight preprocessing
# ----------------------------------------------------------------------------

def _precompute_P(Wh, bh, Wp, Wf):
    """Collapse past/future dense structure into P[(i, m)] (fp64)."""
    Wh = Wh.astype(np.float64)
    Wp = Wp.astype(np.float64)
    Wf = Wf.astype(np.float64)
    nl = L
    Z = np.zeros((H, H))
    S = {}
    for k in range(nl):
        for i in range(nl):
            S[(k, i)] = sum((Wf[k * (nl - 1) + (j - 1)] for j in range(i + 1, nl)), start=Z)
    G = {(0, 0): np.eye(H)}
    for i in range(1, nl):
        G[(i, i)] = np.eye(H) + S[(i, i)] if i < nl - 1 else np.eye(H)
        for m in range(i):
            G[(i, m)] = sum((S[(k, i)] @ G[(k, m)] for k in range(m, i)), start=Z)
    P = {}
    for i in range(1, nl):
        C = {j: Wh[i - 1] @ Wp[j * (nl - 1) + (i - 1)] for j in range(i)}
        for m in range(i):
            P[(i, m)] = sum((C[j] @ G[(j, m)] for j in range(m, i)), start=Z)
    return P


PAIR_INDEX = {}
for _i in range(1, L):
    for _m in range(_i):
        PAIR_INDEX[(_i, _m)] = len(PAIR_INDEX)


def _pack_weights(W0, b0, Wh, bh, Wp, Wf, Wout, bout):
    P = _precompute_P(Wh, bh, Wp, Wf)
    # W0.T in K-blocks padded 112 -> 128 rows: [128, 7, 64]
    w0t = np.zeros((KP, KBLK, H), np.float16)
    w0t[:112] = np.ascontiguousarray(
        W0.astype(np.float64).T.reshape(KBLK, 112, H).transpose(1, 0, 2)
    ).astype(np.float16)
    # P[i][m].T duplicated into both partition halves: [28, 128, 64]
    wpd = np.zeros((NPAIR, 128, H), np.float16)
    for (i, m), k in PAIR_INDEX.items():
        pt = P[(i, m)].T.astype(np.float16)
        wpd[k, 0:H] = pt
        wpd[k, H:128] = pt
    woutt_d = np.zeros((128, OUT), np.float16)
    woutt_d[0:H] = Wout.T.astype(np.float16)
    woutt_d[H:128] = Wout.T.astype(np.float16)
    # per-layer biases duplicated into both halves: [128, 8]
    bias8 = np.zeros((128, L), np.float32)
    bias8[0:H, 0] = b0
    bias8[H:128, 0] = b0
    for i in range(1, L):
        bias8[0:H, i] = bh[i - 1]
        bias8[H:128, i] = bh[i - 1]
    # bout broadcast over the transpose blocks: [128, 160]
    boutb = np.tile(bout.astype(np.float32), (128, NBLK_MAX))
    ident = np.eye(OUT, dtype=np.float32)
    return dict(
        w0t=w0t, wpd=wpd, woutt_d=woutt_d, bias8=bias8, boutb=boutb, ident=ident
    )


# ----------------------------------------------------------------------------
# Device program
# ----------------------------------------------------------------------------

def build_nc(sched=None, silu_via_sigmoid=False):
    nc = bass.Bass()
    sched = SCHED if sched is None else sched
    b_core = sum(m["mega"] for m in sched)
    acols = sum(m["nblk"] * OUT for m in sched)

    xoffs, xcols = _xoff(sched)
    xt_e = nc.dram_tensor("xt", [KP, xcols], f16, kind="ExternalInput")
    w0t_e = nc.dram_tensor("w0t", [KP, KBLK, H], f16, kind="ExternalInput")
    wpd_e = nc.dram_tensor("wpd", [NPAIR, 128, H], f16, kind="ExternalInput")
    woutt_e = nc.dram_tensor("woutt_d", [128, OUT], f16, kind="ExternalInput")
    bias8_e = nc.dram_tensor("bias8", [128, L], f32, kind="ExternalInput")
    boutb_e = nc.dram_tensor("boutb", [128, OUT * NBLK_MAX], f32, kind="ExternalInput")
    ident_e = nc.dram_tensor("ident", [OUT, OUT], f32, kind="ExternalInput")
    o_e = nc.dram_tensor("o", [128, acols], f32, kind="ExternalOutput")

    with tile.TileContext(nc) as tc, ExitStack() as ctx:
        consts = ctx.enter_context(tc.tile_pool(name="consts", bufs=1))
        xpool = ctx.enter_context(tc.tile_pool(name="xpool", bufs=6))
        tpool = ctx.enter_context(tc.tile_pool(name="tpool", bufs=1))
        lpool = ctx.enter_context(tc.tile_pool(name="lpool", bufs=2))
        apool = ctx.enter_context(tc.tile_pool(name="apool", bufs=1))
        pp = ctx.enter_context(tc.tile_pool(name="pp", bufs=3, space="PSUM"))
        p2 = ctx.enter_context(tc.tile_pool(name="p2", bufs=2, space="PSUM"))

        # constants; order matters — the first megatile's x loads go first so
        # PE can start within ~5us, small consts follow
        w0t_s = consts.tile([KP, KBLK, H], f16)
        wpd_s = consts.tile([128, NPAIR, H], f16)
        woutt_s = consts.tile([128, OUT], f16)
        bias_s = consts.tile([128, L], f32)
        boutb_s = consts.tile([128, OUT * NBLK_MAX], f32)
        ident_s = consts.tile([OUT, OUT], f32)

        xts = {}

        def load_xts(m):
            # one DMA per (megatile, chunk): 128 maximal descriptors; chunk A
            # rides the SP HWDGE ring, chunk B the ACT ring
            mg, chunk = m["mg"], m["chunk"]
            for ck in range(2):
                xc = xpool.tile(
                    [KP, KBLK, chunk], f16, tag="xts", name=f"x{mg}{ck}"
                )
                eng = nc.sync if ck == 0 else nc.scalar
                off = xoffs[(mg, ck)]
                eng.dma_start(
                    xc[:],
                    xt_e[:, off : off + KBLK * chunk].rearrange(
                        "p (j c) -> p j c", j=KBLK
                    ),
                )
                xts[(mg, ck)] = xc

        nc.sync.dma_start(w0t_s[:], w0t_e[:])
        for _m in sched[: min(2, len(sched))]:
            load_xts(_m)
        nc.sync.dma_start(bias_s[:], bias8_e[:])
        nc.sync.dma_start(wpd_s[:], wpd_e[:].rearrange("k p m -> p k m"))
        nc.sync.dma_start(woutt_s[:], woutt_e[:])
        nc.sync.dma_start(boutb_s[:], boutb_e[:])
        nc.sync.dma_start(ident_s[:], ident_e[:])

        # Prime ACT/DVE vector clocks on the const DMAs so later activation
        # instructions need only a single sync wait (walrus's activation
        # encoding rejects multi-sem waits: "Too many sync wait commands").
        prim_a = consts.tile([128, 1], f32)
        nc.scalar.copy(prim_a[:], bias_s[:, 0:1])
        prim_v = consts.tile([128, 1], f32)
        nc.vector.tensor_copy(prim_v[:], boutb_s[:, 0:1])

        out_acc = apool.tile([128, acols], f32)

        def emit_silu(dst, src, bias_ap):
            # dst = silu(src + bias) = (src + bias) * sigmoid(src + bias)
            if not silu_via_sigmoid:
                nc.scalar.activation(dst, src, AF.Silu, bias=bias_ap)
            else:  # CoreSim lacks Silu; mathematically identical path
                sg = tpool.tile(list(dst.shape), f32, tag="sg", name="sg", bufs=2)
                nc.scalar.activation(sg[:], src, AF.Sigmoid, bias=bias_ap)
                nc.vector.scalar_tensor_tensor(
                    out=dst, in0=src, scalar=bias_ap, in1=sg[:],
                    op0=mybir.AluOpType.add, op1=mybir.AluOpType.mult,
                )

        def col_groups(chunk):
            w = min(chunk, 512)
            return [slice(h * w, (h + 1) * w) for h in range(chunk // w)]

        def emit_l0(m, T):
            # K-source-major emission: both column-halves of a chunk issue
            # back-to-back under one stationary load per array column-group
            mg, chunk = m["mg"], m["chunk"]
            ps = pp.tile([128, chunk], f32, tag="pre", name=f"ps0_{mg}")
            cgs = col_groups(chunk)
            for j in range(KBLK):
                first = j == 0
                last = j == KBLK - 1
                for cs in cgs:
                    nc.tensor.matmul(
                        ps[0:H, cs], w0t_s[:, j, :], xts[(mg, 0)][:, j, cs],
                        start=first, stop=last, skip_group_check=True,
                    )
                for cs in cgs:
                    nc.tensor.matmul(
                        ps[H:128, cs], w0t_s[:, j, :], xts[(mg, 1)][:, j, cs],
                        start=first, stop=last, skip_group_check=True,
                    )
            emit_silu(T[0][:], ps[:], bias_s[:, 0:1])

        def emit_dense(m, T, i):
            mg, chunk = m["mg"], m["chunk"]
            ps = pp.tile([128, chunk], f32, tag="pre", name=f"ps{i}_{mg}")
            cgs = col_groups(chunk)
            for mm in range(i):
                k = PAIR_INDEX[(i, mm)]
                first = mm == 0
                last = mm == i - 1
                for cs in cgs:
                    nc.tensor.matmul(
                        ps[0:H, cs], wpd_s[0:H, k, :], T[mm][0:H, cs],
                        start=first, stop=last, skip_group_check=True,
                    )
                for cs in cgs:
                    nc.tensor.matmul(
                        ps[H:128, cs], wpd_s[H:128, k, :], T[mm][H:128, cs],
                        start=first, stop=last, skip_group_check=True,
                    )
            emit_silu(T[i][:], ps[:], bias_s[:, i : i + 1])

        def emit_logits(m, T):
            mg, chunk = m["mg"], m["chunk"]
            lgsT = lpool.tile([OUT, m["mega"]], f32, tag="lgsT", name=f"lg{mg}")
            for ck in range(2):
                for hi, cs in enumerate(col_groups(chunk)):
                    w = cs.stop - cs.start
                    plg = pp.tile([OUT, w], f32, tag="pre", name=f"plg{mg}")
                    nc.tensor.matmul(
                        plg[:],
                        woutt_s[ck * H : ck * H + H, :],
                        T[L - 1][ck * H : ck * H + H, cs],
                        start=True, stop=True,
                    )
                    seg = ck * chunk + cs.start
                    nc.vector.tensor_copy(lgsT[:, seg : seg + w], plg[:])
            return lgsT

        def emit_transpose(m, lgsT):
            mg, nblk = m["mg"], m["nblk"]
            pt = p2.tile([128, OUT * nblk], f32, tag="pt", name=f"pt{mg}")
            for blk in range(nblk):
                nc.tensor.matmul(
                    pt[:, blk * OUT : (blk + 1) * OUT],
                    lgsT[:, blk * 128 : (blk + 1) * 128],
                    ident_s[:],
                    is_transpose=True,
                    start=True, stop=True, skip_group_check=True,
                )
            nc.vector.tensor_add(
                out_acc[:, m["aoff"] : m["aoff"] + OUT * nblk],
                pt[:],
                boutb_s[:, 0 : OUT * nblk],
            )

        # Two megatiles in flight, layer-interleaved: megatile B's matmuls
        # cover megatile A's silu latency so PE never drains (HAM stays warm).
        # The next pair's layer-0 is emitted before this pair's transposes so
        # PE has dense work while DVE stages the logits for transposition.
        pairs = [sched[i : i + 2] for i in range(0, len(sched), 2)]

        def alloc_T(m):
            mg = m["mg"]
            return [
                tpool.tile(
                    [128, m["chunk"]], f16, tag=f"T{i}_{mg % 2}", name=f"T{i}_{mg}"
                )
                for i in range(L)
            ]

        # ---- deferred log-softmax over groups of 10, batch-major ----
        ngrp = acols // OUT
        ex = apool.tile([128, acols], f32)
        sm = apool.tile([128, ngrp], f32)
        lsm = apool.tile([128, ngrp], f32)
        od = apool.tile([128, acols], f32)

        def emit_softmax(c0, c1):
            g0, g1 = c0 // OUT, c1 // OUT
            nc.scalar.activation(ex[:, c0:c1], out_acc[:, c0:c1], AF.Exp)
            nc.vector.reduce_sum(
                out=sm[:, g0:g1],
                in_=ex[:, c0:c1].rearrange("p (g c) -> p g c", c=OUT),
                axis=mybir.AxisListType.X,
            )
            nc.scalar.activation(lsm[:, g0:g1], sm[:, g0:g1], AF.Ln)
            for c in range(OUT):
                nc.vector.tensor_sub(
                    od[:, c0:c1].rearrange("p (g c) -> p g c", c=OUT)[:, :, c],
                    out_acc[:, c0:c1].rearrange("p (g c) -> p g c", c=OUT)[:, :, c],
                    lsm[:, g0:g1],
                )
            nc.sync.dma_start(o_e[:, c0:c1], od[:, c0:c1])

        last_aoff = sched[-2]["aoff"] if len(sched) > 1 else 0

        Ts = {m["mg"]: alloc_T(m) for m in pairs[0]}
        for m in pairs[0]:
            emit_l0(m, Ts[m["mg"]])
        for pr, mgs in enumerate(pairs):
            last_pair = pr == len(pairs) - 1
            nxt = pairs[pr + 1] if pr + 1 < len(pairs) else []
            for m in nxt:
                if (m["mg"], 0) not in xts:
                    load_xts(m)
            for i in range(1, L):
                for m in mgs:
                    emit_dense(m, Ts[m["mg"]], i)
                if last_pair and i == 4 and last_aoff > 0:
                    # softmax of all finished megatiles, hidden under this
                    # pair's remaining dense layers (costs 2 ACT table swaps)
                    emit_softmax(0, last_aoff)
            lgs = {m["mg"]: emit_logits(m, Ts[m["mg"]]) for m in mgs}
            for m in nxt:
                Ts[m["mg"]] = alloc_T(m)
                emit_l0(m, Ts[m["mg"]])
            for m in mgs:
                emit_transpose(m, lgs[m["mg"]])

        emit_softmax(last_aoff, acols)

    _split_multi_waits(nc)
    return nc


def _split_multi_waits(nc):
    """walrus's activation encoding admits one sync-wait; hoist extras onto
    preceding same-engine NoOps (sequentially equivalent)."""
    for blk in nc.m.functions[0].blocks:
        idx = 0
        while idx < len(blk.instructions):
            inst = blk.instructions[idx]
            si = inst.sync_info
            splittable = isinstance(
                inst,
                (
                    mybir.InstActivation,
                    mybir.InstTensorCopy,
                    mybir.InstTensorTensor,
                    mybir.InstTensorReduce,
                    mybir.InstMatmult,
                    mybir.InstLdweights,
                    mybir.InstDMACopy,
                    mybir.InstMemset,
                    mybir.InstDrain,
                    mybir.InstStreamTranspose,
                ),
            )
            if splittable and si is not None and len(si.on_wait) > 1:
                extras = list(si.on_wait[:-1])
                si.on_wait = [si.on_wait[-1]]
                for w in reversed(extras):
                    nop = mybir.InstNoOp(
                        name=nc.get_next_instruction_name(), ins=[], outs=[]
                    )
                    nop.engine = inst.engine
                    nop.sync_info = mybir.SyncInfo(on_wait=[w], on_update=[])
                    nc.register_instruction(nop)
                    blk.instructions.insert(idx, nop)
                    idx += 1
            idx += 1


# ----------------------------------------------------------------------------
# Host wrapper
# ----------------------------------------------------------------------------

_CACHE = {}


def _get_nc():
    if "nc" not in _CACHE:
        _CACHE["nc"] = build_nc()
    return _CACHE["nc"]


def pack_x(x_slice, sched=None):
    """[rows, 784] fp32 -> per-core tiled layout [128, XCOLS] fp16: slab per
    (megatile, chunk), per-partition contiguous [KBLK, chunk] blocks."""
    sched = SCHED if sched is None else sched
    xoffs, xcols = _xoff(sched)
    xt16 = np.zeros((KBLK, KP, x_slice.shape[0]), np.float16)
    xt16[:, :112, :] = x_slice.T.astype(np.float16).reshape(KBLK, 112, -1)
    out = np.empty((KP, xcols), np.float16)
    for m in sched:
        for ck in range(2):
            lo = m["start"] + ck * m["chunk"]
            off = xoffs[(m["mg"], ck)]
            blk = xt16[:, :, lo : lo + m["chunk"]].transpose(1, 0, 2)
            out[:, off : off + KBLK * m["chunk"]] = blk.reshape(KP, -1)
    return out


def prepare_inputs(x, W0, b0, Wh, bh, Wp, Wf, Wout, bout):
    consts = _pack_weights(W0, b0, Wh, bh, Wp, Wf, Wout, bout)
    in_maps = []
    for c in range(N_CORES):
        m = dict(consts)
        m["xt"] = pack_x(x[c * B_CORE : (c + 1) * B_CORE])
        in_maps.append(m)
    return in_maps


def _unpermute(o_core, sched=None):
    sched = SCHED if sched is None else sched
    b_core = sum(m["mega"] for m in sched)
    out = np.empty((b_core, OUT), np.float32)
    for m in sched:
        seg = o_core[:, m["aoff"] : m["aoff"] + m["nblk"] * OUT]
        seg = seg.reshape(128, m["nblk"], OUT).transpose(1, 0, 2)
        out[m["start"] : m["start"] + m["mega"]] = seg.reshape(m["mega"], OUT)
    return out


def run(inputs, trace=False, **kw):
    in_maps = prepare_inputs(**inputs)
    nc = _get_nc()
    res = run_bass_kernel_spmd(nc, in_maps, list(range(N_CORES)), trace=trace, **kw)
    out = np.empty((B, OUT), np.float32)
    for c in range(N_CORES):
        out[c * B_CORE : (c + 1) * B_CORE] = _unpermute(res.results[c]["o"])
    return out, res


def kernel(**inputs):
    out, _ = run(inputs, trace=False)
    return out
